# revision 1
# baseline (speedup 1.0000x reference)
"""Two-layer GraphSAGE (ClusterGCN-style) on 8 Trainium2 NeuronCores.

Strategy (pull-based, dst-partitioned):
  - Host: balance nodes into 128-node "windows" (8 cores x n_win windows)
    via LPT on degree so every window has ~E/784 incident edges.  Edges are
    grouped by (destination window, source range) where the source table is
    split into 4 ranges of <=32768 rows (dma_gather indices are int16).
    Groups are padded to whole 128-edge tiles with cross-core-uniform tile
    counts; per-edge inverse-degree is folded into the selection ("one-hot")
    matrix, with padding edges given weight 0.
  - Device, layer 1: dma_gather of x[src] rows (fp16 table, replicated in
    every core's HBM) into [128 edges x 128 feat] tiles; a PE matmul with a
    DVE-built one-hot turns the gather into a segmented mean:
    psumA[c, n] = sum_e msg[e, c] * onehot[e, n].  Then
    hT[h, n] = Wl1^T @ mean + Wr1^T @ xT + bl1, ReLU, kept in SBUF (fp16).
  - Exchange: p = h @ Wl2 computed per-core, AllGather'd (fp16, 64 cols),
    then pad-expanded to a 256B-row table for gathering.
  - Device, layer 2: same gather/one-hot aggregation over p rows plus the
    self term hT^T @ Wr2 and bias, then a fused log_softmax epilogue.
  - Host: inverse-permute the per-core outputs back to original node order.
"""

import sys

for _p in ("/opt/trn_rl_repo",):
    if _p not in sys.path:
        sys.path.insert(0, _p)

import heapq

import numpy as np

import concourse.bacc as bacc
import concourse.bass as bass
import concourse.mybir as mybir
import concourse.tile as tile
from concourse.bass_utils import run_bass_kernel_spmd

F16 = mybir.dt.float16
F32 = mybir.dt.float32
I16 = mybir.dt.int16
AX = mybir.AxisListType
ALU = mybir.AluOpType
ACTF = mybir.ActivationFunctionType

N_CORES = 8
P = 128
R = 4  # source-table ranges (int16 index limit)
CHUNK_WINDOWS = 7


class LayerPlan:
    pass


class Prep:
    pass


# ---------------------------------------------------------------------------
# Host-side prep
# ---------------------------------------------------------------------------
def _layer_plan(key, e_win_sorted, e_dst, slot_in_win, invdeg, n_rows, n_win):
    """Build the tile schedule + per-core slot arrays for one layer.

    key: per-edge source row in the gather table (edges already sorted by
    destination window).  Returns a LayerPlan with cross-core-uniform
    structure and per-core data arrays.
    """
    n_windows = N_CORES * n_win
    rs = -(-n_rows // R)  # range size
    assert rs <= 32768
    r_e = (key // rs).astype(np.int64)

    cnt = np.zeros((n_windows, R), dtype=np.int64)
    np.add.at(cnt, (e_win_sorted, r_e), 1)
    # cross-core-uniform tiles per (window position, range)
    tiles_cw = -(-cnt // P).reshape(N_CORES, n_win, R)
    tiles_pos = tiles_cw.max(axis=0)  # [n_win, R]

    # chunk schedule: windows in chunks of CHUNK_WINDOWS; per chunk the
    # buffer holds, for r in range(R), for w in chunk, tiles_pos[w, r] tiles.
    chunks = []
    base_tile = np.zeros((n_win, R), dtype=np.int64)  # global tile base per (w,r)
    nt_total = 0
    for w0 in range(0, n_win, CHUNK_WINDOWS):
        w1 = min(w0 + CHUNK_WINDOWS, n_win)
        ch = {"w0": w0, "w1": w1, "tile0": nt_total, "calls": []}
        t = nt_total
        for r in range(R):
            s0 = t
            for w in range(w0, w1):
                base_tile[w, r] = t
                t += int(tiles_pos[w, r])
            ch["calls"].append((r, s0 - nt_total, t - nt_total))  # rel tile range
        ch["nt"] = t - nt_total
        # per window: list of buffer-relative tile indices (in r, t order)
        ch["windows"] = []
        for w in range(w0, w1):
            tl = []
            for r in range(R):
                b = base_tile[w, r] - nt_total
                tl.extend(range(int(b), int(b + tiles_pos[w, r])))
            ch["windows"].append((w, tl))
        nt_total = t
        chunks.append(ch)

    # per-edge slot assignment (slot ids are per-core, same layout each core)
    order2 = np.lexsort((r_e, e_win_sorted))
    w2 = e_win_sorted[order2]
    r2 = r_e[order2]
    k2 = key[order2]
    d2 = e_dst[order2]
    # cumcount within (w, r) group
    grp = w2 * R + r2
    grp_start = np.zeros(n_windows * R + 1, dtype=np.int64)
    np.cumsum(np.bincount(grp, minlength=n_windows * R), out=grp_start[1:])
    cumcount = np.arange(len(w2), dtype=np.int64) - grp_start[grp]
    w_pos = w2 % n_win
    e_core = w2 // n_win
    slot = base_tile[w_pos, r2] * P + cumcount

    # per-core flat arrays [nt_total * P]
    dstl = np.zeros((N_CORES, nt_total * P), dtype=np.float32)
    ivd = np.zeros((N_CORES, nt_total * P), dtype=np.float32)
    kf = np.zeros((N_CORES, nt_total * P), dtype=np.int16)
    flat = e_core * (nt_total * P) + slot
    dstl.reshape(-1)[flat] = slot_in_win[d2].astype(np.float32)
    ivd.reshape(-1)[flat] = invdeg[d2].astype(np.float32)
    kf.reshape(-1)[flat] = (k2 - r2 * rs).astype(np.int16)

    # dstl/ivd SBUF layout [P, nt_total]: [p, col] = slot col*P + p
    dstl = np.ascontiguousarray(dstl.reshape(N_CORES, nt_total, P).transpose(0, 2, 1))
    ivd = np.ascontiguousarray(ivd.reshape(N_CORES, nt_total, P).transpose(0, 2, 1))
    # idx blob [P, nt_total*8]: wrapped 16-partition layout, replicated x8;
    # call (tile_s0, tile_s1) -> cols [tile_s0*8, tile_s1*8)
    kw = kf.reshape(N_CORES, nt_total * P // 16, 16).transpose(0, 2, 1)  # [C,16,F]
    idx = np.ascontiguousarray(np.tile(kw, (1, 8, 1)))  # [C, 128, F]

    pl = LayerPlan()
    pl.rs = rs
    pl.nt = nt_total
    pl.chunks = chunks
    pl.dstl = dstl
    pl.ivd = ivd
    pl.idx = idx
    return pl


def _host_prep(x, edge_index, n_win):
    n_nodes = x.shape[0]
    src = np.asarray(edge_index[0], dtype=np.int64)
    dst = np.asarray(edge_index[1], dtype=np.int64)

    deg = np.bincount(dst, minlength=n_nodes).astype(np.int64)
    invdeg = (1.0 / np.maximum(deg, 1)).astype(np.float32)

    n_windows = N_CORES * n_win
    slots_per_core = n_win * P
    total_slots = N_CORES * slots_per_core
    assert total_slots >= n_nodes

    # LPT: nodes sorted by degree desc -> least-loaded window with capacity
    order = np.argsort(-deg, kind="stable")
    win_of = np.empty(n_nodes, dtype=np.int64)
    slot_in_win = np.empty(n_nodes, dtype=np.int64)
    heap = [(0, w) for w in range(n_windows)]
    heapq.heapify(heap)
    counts = np.zeros(n_windows, dtype=np.int64)
    for node in order:
        while True:
            sld, w = heapq.heappop(heap)
            if counts[w] < P:
                break
        win_of[node] = w
        slot_in_win[node] = counts[w]
        counts[w] += 1
        if counts[w] < P:
            heapq.heappush(heap, (sld + int(deg[node]), w))

    # relabel windows within each core so tile profiles align across cores:
    # tiles-per-position is a cross-core max, so sorting windows by their
    # (layer-1 range, layer-2 range) tile vectors keeps that max tight.
    rs1 = -(-n_nodes // R)
    cnt1 = np.zeros((n_windows, R), dtype=np.int64)
    np.add.at(cnt1, (win_of[dst], src // rs1), 1)
    core_of = win_of // n_win
    cnt2 = np.zeros((n_windows, R), dtype=np.int64)
    np.add.at(cnt2, (win_of[dst], core_of[src] // 2), 1)
    prof = np.concatenate([-(-cnt1 // P), -(-cnt2 // P)], axis=1)
    win_pos = np.empty(n_windows, dtype=np.int64)
    for c in range(N_CORES):
        ws = np.arange(c * n_win, (c + 1) * n_win)
        keys = tuple(prof[ws, k] for k in range(prof.shape[1] - 1, -1, -1))
        srt = np.lexsort(keys)
        rank = np.empty(n_win, dtype=np.int64)
        rank[srt] = np.arange(n_win)
        win_pos[ws] = c * n_win + (n_win - 1 - rank)
    win_of = win_pos[win_of]

    node_core = win_of // n_win
    node_lpos = (win_of % n_win) * P + slot_in_win
    node_gslot = node_core * slots_per_core + node_lpos

    # edges sorted by destination window
    e_win = win_of[dst]
    e_order = np.argsort(e_win, kind="stable")
    e_src = src[e_order]
    e_dst = dst[e_order]
    e_win_sorted = e_win[e_order]

    l1 = _layer_plan(e_src, e_win_sorted, e_dst, slot_in_win, invdeg,
                     n_nodes, n_win)
    l2 = _layer_plan(node_gslot[e_src], e_win_sorted, e_dst, slot_in_win,
                     invdeg, total_slots, n_win)

    # xT per core: [in_c, slots_per_core] f32, pad slots -> 0
    in_c = x.shape[1]
    xT = np.zeros((N_CORES, in_c, slots_per_core), dtype=np.float32)
    xx = np.asarray(x, dtype=np.float32)
    for c in range(N_CORES):
        nodes_c = np.nonzero(node_core == c)[0]
        xT[c][:, node_lpos[nodes_c]] = xx[nodes_c].T

    prep = Prep()
    prep.n_win = n_win
    prep.total_slots = total_slots
    prep.slots_per_core = slots_per_core
    prep.l1 = l1
    prep.l2 = l2
    prep.xT = xT
    prep.node_core = node_core
    prep.node_lpos = node_lpos
    return prep


# ---------------------------------------------------------------------------
# Device program
# ---------------------------------------------------------------------------
def _aggregate_layer(nc, tc, pl, tab_ap, elem, rhs_cols, iota_t, dstl_t, ivd_t,
                     idx_d, window_body, prefix):
    """Emit gather + one-hot aggregation for one layer.

    For each window, window_body(w, agg_emit) is called; agg_emit(psum_t,
    rhs_of_tile) emits one one-hot build + one matmul per tile of the
    window, calling rhs_of_tile(psum_t, msg_slice, oh_t, first, last).
    `elem` is the gathered row width (fp16 elements, 256B-aligned);
    `rhs_cols` is how many of those columns the matmul consumes.
    """
    nt_max = max(ch["nt"] for ch in pl.chunks)
    with (
        tc.tile_pool(name=f"{prefix}idx", bufs=2) as idxp,
        tc.tile_pool(name=f"{prefix}msg", bufs=2) as msgp,
        tc.tile_pool(name=f"{prefix}oh", bufs=4) as ohp,
    ):
        for ch in pl.chunks:
            nt = ch["nt"]
            t0 = ch["tile0"]
            f0, f1 = t0 * 8, (t0 + nt) * 8
            idx_t = idxp.tile([P, nt_max * 8], I16, tag="idx")
            nc.sync.dma_start(idx_t[:, : f1 - f0], idx_d[:, f0:f1])
            msg_t = msgp.tile([P, nt_max, elem], F16, tag="msg")
            for r, ts0, ts1 in ch["calls"]:
                if ts1 == ts0:
                    continue
                ni = (ts1 - ts0) * P
                nc.gpsimd.dma_gather(
                    msg_t[:, ts0:ts1, :],
                    tab_ap[r * pl.rs :, :],
                    idx_t[:, ts0 * 8 : ts1 * 8],
                    ni,
                    ni,
                    elem,
                    single_packet=False,
                )
            for w, tl in ch["windows"]:

                def agg_emit(psum_t, rhs_of_tile, tl=tl, t0=t0, msg_t=msg_t):
                    for k, bi in enumerate(tl):
                        col = t0 + bi
                        oh_t = ohp.tile([P, P], F16, tag="oh")
                        nc.vector.tensor_scalar(
                            out=oh_t[:],
                            in0=iota_t[:],
                            scalar1=dstl_t[:, col : col + 1],
                            scalar2=ivd_t[:, col : col + 1],
                            op0=ALU.is_equal,
                            op1=ALU.mult,
                        )
                        rhs_of_tile(
                            psum_t, msg_t[:, bi, :rhs_cols], oh_t,
                            k == 0, k == len(tl) - 1,
                        )

                window_body(w, agg_emit)


def _build_program(n_nodes, in_c, hid_c, out_c, n_win, prep, single_core=False):
    l1, l2 = prep.l1, prep.l2
    total_slots = prep.total_slots
    slots_per_core = prep.slots_per_core

    nc = bacc.Bacc(
        "TRN2",
        target_bir_lowering=False,
        debug=False,
        num_devices=1 if single_core else N_CORES,
    )

    x16_d = nc.dram_tensor("x16", [n_nodes, in_c], F16, kind="ExternalInput")
    xT_d = nc.dram_tensor("xT", [in_c, slots_per_core], F32, kind="ExternalInput")
    idx1_d = nc.dram_tensor("idx1", [P, l1.nt * 8], I16, kind="ExternalInput")
    idx2_d = nc.dram_tensor("idx2", [P, l2.nt * 8], I16, kind="ExternalInput")
    dstl1_d = nc.dram_tensor("dstl1", [P, l1.nt], F32, kind="ExternalInput")
    ivd1_d = nc.dram_tensor("ivd1", [P, l1.nt], F32, kind="ExternalInput")
    dstl2_d = nc.dram_tensor("dstl2", [P, l2.nt], F32, kind="ExternalInput")
    ivd2_d = nc.dram_tensor("ivd2", [P, l2.nt], F32, kind="ExternalInput")
    wl1_d = nc.dram_tensor("wl1", [in_c, hid_c], F16, kind="ExternalInput")
    wr1_d = nc.dram_tensor("wr1", [in_c, hid_c], F32, kind="ExternalInput")
    bl1_d = nc.dram_tensor("bl1", [1, hid_c], F32, kind="ExternalInput")
    wl2_d = nc.dram_tensor("wl2", [hid_c, out_c], F16, kind="ExternalInput")
    wr2_d = nc.dram_tensor("wr2", [hid_c, out_c], F16, kind="ExternalInput")
    bl2_d = nc.dram_tensor("bl2", [1, out_c], F32, kind="ExternalInput")
    iota_d = nc.dram_tensor("iota", [P, P], F16, kind="ExternalInput")
    out_d = nc.dram_tensor("out", [slots_per_core, out_c], F32, kind="ExternalOutput")

    with tile.TileContext(nc) as tc:
        with (
            tc.tile_pool(name="const", bufs=1) as constp,
            tc.tile_pool(name="persist", bufs=1) as persist,
            tc.tile_pool(name="dram", bufs=1, space="DRAM") as dramp,
        ):
            iota_t = constp.tile([P, P], F16)
            nc.sync.dma_start(iota_t[:], iota_d[:])
            wl1_t = constp.tile([in_c, hid_c], F16)
            nc.sync.dma_start(wl1_t[:], wl1_d[:])
            wr1_t = constp.tile([in_c, hid_c], F32)
            nc.sync.dma_start(wr1_t[:], wr1_d[:])
            bl1_t = constp.tile([1, hid_c], F32)
            nc.sync.dma_start(bl1_t[:], bl1_d[:])
            wl2_t = constp.tile([hid_c, out_c], F16)
            nc.sync.dma_start(wl2_t[:], wl2_d[:])
            wr2_t = constp.tile([hid_c, out_c], F16)
            nc.sync.dma_start(wr2_t[:], wr2_d[:])
            bl2_t = constp.tile([1, out_c], F32)
            nc.sync.dma_start(bl2_t[:], bl2_d[:])
            ones_t = constp.tile([1, P], F32)
            nc.vector.memset(ones_t[:], 1.0)

            dstl1_t = persist.tile([P, l1.nt], F32)
            nc.sync.dma_start(dstl1_t[:], dstl1_d[:])
            ivd1_t = persist.tile([P, l1.nt], F32)
            nc.sync.dma_start(ivd1_t[:], ivd1_d[:])
            dstl2_t = persist.tile([P, l2.nt], F32)
            nc.sync.dma_start(dstl2_t[:], dstl2_d[:])
            ivd2_t = persist.tile([P, l2.nt], F32)
            nc.sync.dma_start(ivd2_t[:], ivd2_d[:])
            hT_all = persist.tile([hid_c, n_win * P], F16)
            z_all = persist.tile([P, n_win * out_c], F32)
            mx_all = persist.tile([P, n_win], F32)
            se_all = persist.tile([P, n_win], F32)
            ls_all = persist.tile([P, n_win], F32)

            cc_in = dramp.tile([slots_per_core, out_c], F16)
            cc_out = dramp.tile([total_slots, out_c], F16)
            p_pad = dramp.tile([total_slots, P], F16)

            # ---------------- Layer 1 ----------------
            with (
                tc.tile_pool(name="xt", bufs=2) as xtp,
                tc.tile_pool(name="mean", bufs=2) as meanp,
                tc.tile_pool(name="psA", bufs=2, space="PSUM") as psA,
                tc.tile_pool(name="psB", bufs=2, space="PSUM") as psB,
            ):
                xt_tiles = {}

                def l1_body(w, agg_emit):
                    psa_t = psA.tile([in_c, P], F32, space="PSUM", tag="psa")

                    def rhs1(psum_t, msg_ap, oh_t, first, last):
                        nc.tensor.matmul(
                            out=psum_t[:], lhsT=msg_ap, rhs=oh_t[:],
                            start=first, stop=last,
                        )

                    agg_emit(psa_t, rhs1)
                    mean_t = meanp.tile([in_c, P], F16, tag="mean")
                    nc.vector.tensor_copy(out=mean_t[:], in_=psa_t[:])
                    if w % CHUNK_WINDOWS == 0:
                        wn = min(CHUNK_WINDOWS, n_win - w)
                        xt_t = xtp.tile([in_c, CHUNK_WINDOWS * P], F32, tag="xt")
                        nc.sync.dma_start(
                            xt_t[:, : wn * P], xT_d[:, w * P : (w + wn) * P]
                        )
                        xt_tiles[w] = xt_t
                    xt_t = xt_tiles[w - w % CHUNK_WINDOWS]
                    wi = w % CHUNK_WINDOWS
                    psb_t = psB.tile([hid_c, P], F32, space="PSUM", tag="psb")
                    nc.tensor.matmul(
                        out=psb_t[:], lhsT=wl1_t[:], rhs=mean_t[:],
                        start=True, stop=False,
                    )
                    nc.tensor.matmul(
                        out=psb_t[:], lhsT=wr1_t[:],
                        rhs=xt_t[:, wi * P : (wi + 1) * P],
                        start=False, stop=False,
                    )
                    nc.tensor.matmul(
                        out=psb_t[:], lhsT=bl1_t[:], rhs=ones_t[:],
                        start=False, stop=True,
                    )
                    nc.vector.tensor_scalar(
                        out=hT_all[:, w * P : (w + 1) * P],
                        in0=psb_t[:],
                        scalar1=0.0,
                        scalar2=None,
                        op0=ALU.max,
                    )

                _aggregate_layer(
                    nc, tc, l1, x16_d[:], in_c, in_c, iota_t, dstl1_t, ivd1_t,
                    idx1_d, l1_body, "g1",
                )

            # ---------------- transform + AllGather + pad-expand ----------
            with (
                tc.tile_pool(name="pp", bufs=3) as ppp,
                tc.tile_pool(name="psP", bufs=2, space="PSUM") as psP,
            ):
                for w in range(n_win):
                    psp_t = psP.tile([P, out_c], F32, space="PSUM", tag="psp")
                    nc.tensor.matmul(
                        out=psp_t[:],
                        lhsT=hT_all[:, w * P : (w + 1) * P],
                        rhs=wl2_t[:],
                        start=True,
                        stop=True,
                    )
                    p_t = ppp.tile([P, out_c], F16, tag="pp")
                    nc.vector.tensor_copy(out=p_t[:], in_=psp_t[:])
                    nc.sync.dma_start(cc_in[w * P : (w + 1) * P, :], p_t[:])
                if not single_core:
                    nc.gpsimd.collective_compute(
                        "AllGather",
                        ALU.bypass,
                        replica_groups=[list(range(N_CORES))],
                        ins=[cc_in[:].opt()],
                        outs=[cc_out[:].opt()],
                    )
                # pad-expand in row chunks (single-dim DMA count is 16-bit)
                qs = -(-total_slots // 4)
                for q0 in range(0, total_slots, qs):
                    q1 = min(q0 + qs, total_slots)
                    nc.sync.dma_start(p_pad[q0:q1, :out_c], cc_out[q0:q1, :])

            # ---------------- Layer 2 ----------------
            with (
                tc.tile_pool(name="sm", bufs=4) as smp,
                tc.tile_pool(name="outp", bufs=3) as outp,
                tc.tile_pool(name="ps2", bufs=2, space="PSUM") as ps2,
            ):

                def l2_body(w, agg_emit):
                    ps2_t = ps2.tile([P, out_c], F32, space="PSUM", tag="ps2")

                    def rhs2(psum_t, msg_ap, oh_t, first, last):
                        nc.tensor.matmul(
                            out=psum_t[:], lhsT=oh_t[:], rhs=msg_ap,
                            start=first, stop=False,
                        )

                    agg_emit(ps2_t, rhs2)
                    nc.tensor.matmul(
                        out=ps2_t[:],
                        lhsT=hT_all[:, w * P : (w + 1) * P],
                        rhs=wr2_t[:],
                        start=False,
                        stop=False,
                    )
                    nc.tensor.matmul(
                        out=ps2_t[:], lhsT=ones_t[:], rhs=bl2_t[:],
                        start=False, stop=True,
                    )
                    nc.vector.reduce_max(
                        out=mx_all[:, w : w + 1], in_=ps2_t[:], axis=AX.X,
                        negate=True,
                    )
                    ex_t = smp.tile([P, out_c], F32, tag="ex")
                    nc.scalar.activation(
                        out=ex_t[:], in_=ps2_t[:], func=ACTF.Exp,
                        bias=mx_all[:, w : w + 1], scale=1.0,
                        accum_out=se_all[:, w : w + 1],
                    )
                    nc.vector.tensor_copy(
                        out=z_all[:, w * out_c : (w + 1) * out_c], in_=ps2_t[:]
                    )

                _aggregate_layer(
                    nc, tc, l2, p_pad[:], P, out_c, iota_t, dstl2_t, ivd2_t,
                    idx2_d, l2_body, "g2",
                )
                # batched log(sum(exp)) then final normalize + store
                nc.scalar.activation(out=ls_all[:], in_=se_all[:], func=ACTF.Ln)
                for w in range(n_win):
                    o_t = outp.tile([P, out_c], F32, tag="out")
                    nc.vector.tensor_scalar(
                        out=o_t[:],
                        in0=z_all[:, w * out_c : (w + 1) * out_c],
                        scalar1=mx_all[:, w : w + 1],
                        scalar2=ls_all[:, w : w + 1],
                        op0=ALU.add,
                        op1=ALU.subtract,
                    )
                    nc.sync.dma_start(out_d[w * P : (w + 1) * P, :], o_t[:])

    nc.compile()
    return nc


# ---------------------------------------------------------------------------
# Entry point
# ---------------------------------------------------------------------------
def _run(x, edge_index, Wl1, bl1, Wr1, Wl2, bl2, Wr2, n_win=98, trace=False):
    x = np.asarray(x, dtype=np.float32)
    edge_index = np.asarray(edge_index)
    n_nodes, in_c = x.shape
    hid_c = np.asarray(Wl1).shape[1]
    out_c = np.asarray(Wl2).shape[1]

    prep = _host_prep(x, edge_index, n_win)
    nc = _build_program(n_nodes, in_c, hid_c, out_c, n_win, prep)

    x16 = x.astype(np.float16)
    iota = np.broadcast_to(
        np.arange(P, dtype=np.float16)[None, :], (P, P)
    ).copy()
    shared = {
        "x16": x16,
        "wl1": np.asarray(Wl1, dtype=np.float16),
        "wr1": np.asarray(Wr1, dtype=np.float32),
        "bl1": np.asarray(bl1, dtype=np.float32).reshape(1, -1),
        "wl2": np.asarray(Wl2, dtype=np.float16),
        "wr2": np.asarray(Wr2, dtype=np.float16),
        "bl2": np.asarray(bl2, dtype=np.float32).reshape(1, -1),
        "iota": iota,
    }
    in_maps = []
    for c in range(N_CORES):
        m = dict(shared)
        m["xT"] = prep.xT[c]
        m["idx1"] = prep.l1.idx[c]
        m["idx2"] = prep.l2.idx[c]
        m["dstl1"] = prep.l1.dstl[c]
        m["ivd1"] = prep.l1.ivd[c]
        m["dstl2"] = prep.l2.dstl[c]
        m["ivd2"] = prep.l2.ivd[c]
        in_maps.append(m)

    res = run_bass_kernel_spmd(
        nc, in_maps, core_ids=list(range(N_CORES)), trace=trace
    )

    out = np.empty((n_nodes, out_c), dtype=np.float32)
    for c in range(N_CORES):
        oc = res.results[c]["out"]
        nodes_c = np.nonzero(prep.node_core == c)[0]
        out[nodes_c] = oc[prep.node_lpos[nodes_c]]
    return out, res


def kernel(x, edge_index, Wl1, bl1, Wr1, Wl2, bl2, Wr2):
    out, _ = _run(x, edge_index, Wl1, bl1, Wr1, Wl2, bl2, Wr2)
    return out


def profile_run(np_inputs, n_win=98):
    out, res = _run(n_win=n_win, trace=True, **np_inputs)
    print(f"profile: exec_time_ns={res.exec_time_ns} "
          f"mean={res.mean_exec_time_ns}")
    return res.exec_time_ns



# revision 3
# speedup vs baseline: 1.8605x; 1.8605x over previous
"""Two-layer GraphSAGE (ClusterGCN-style) on 8 Trainium2 NeuronCores.

Strategy (pull-based, dst-partitioned):
  - Host: balance nodes into 128-node "windows" (8 cores x n_win windows)
    via LPT on degree so every window has ~E/784 incident edges.  Edges are
    grouped by (destination window, source range) where the source table is
    split into 4 ranges of <=32768 rows (dma_gather indices are int16).
    Groups are padded to whole 128-edge tiles with cross-core-uniform tile
    counts; per-edge inverse-degree is folded into the selection ("one-hot")
    matrix, with padding edges given weight 0.
  - Device, layer 1: dma_gather of x[src] rows (fp16 table, replicated in
    every core's HBM) into [128 edges x 128 feat] tiles; a PE matmul with a
    DVE-built one-hot turns the gather into a segmented mean:
    psumA[c, n] = sum_e msg[e, c] * onehot[e, n].  Then
    hT[h, n] = Wl1^T @ mean + Wr1^T @ xT + bl1, ReLU, kept in SBUF (fp16).
  - Exchange: p = h @ Wl2 computed per-core, AllGather'd (fp16, 64 cols),
    then pad-expanded to a 256B-row table for gathering.
  - Device, layer 2: same gather/one-hot aggregation over p rows plus the
    self term hT^T @ Wr2 and bias, then a fused log_softmax epilogue.
  - Host: inverse-permute the per-core outputs back to original node order.
"""

import sys

for _p in ("/opt/trn_rl_repo",):
    if _p not in sys.path:
        sys.path.insert(0, _p)

import heapq

import numpy as np

import concourse.bacc as bacc
import concourse.bass as bass
import concourse.mybir as mybir
import concourse.tile as tile
from concourse.bass_utils import run_bass_kernel_spmd

F16 = mybir.dt.float16
F32 = mybir.dt.float32
I16 = mybir.dt.int16
AX = mybir.AxisListType
ALU = mybir.AluOpType
ACTF = mybir.ActivationFunctionType

N_CORES = 8
P = 128
R = 4  # source-table ranges (int16 index limit)
CHUNK_WINDOWS = 7


class LayerPlan:
    pass


class Prep:
    pass


# ---------------------------------------------------------------------------
# Host-side prep
# ---------------------------------------------------------------------------
def _layer_plan(key, e_win_sorted, e_dst, slot_in_win, invdeg, n_rows, n_win):
    """Build the tile schedule + per-core slot arrays for one layer.

    key: per-edge source row in the gather table (edges already sorted by
    destination window).  Returns a LayerPlan with cross-core-uniform
    structure and per-core data arrays.
    """
    n_windows = N_CORES * n_win
    rs = -(-n_rows // R)  # range size
    assert rs <= 32768
    r_e = (key // rs).astype(np.int64)

    cnt = np.zeros((n_windows, R), dtype=np.int64)
    np.add.at(cnt, (e_win_sorted, r_e), 1)
    # cross-core-uniform tiles per (window position, range)
    tiles_cw = -(-cnt // P).reshape(N_CORES, n_win, R)
    tiles_pos = tiles_cw.max(axis=0)  # [n_win, R]

    # chunk schedule: windows in chunks of CHUNK_WINDOWS; per chunk the
    # buffer holds, for r in range(R), for w in chunk, tiles_pos[w, r] tiles.
    chunks = []
    base_tile = np.zeros((n_win, R), dtype=np.int64)  # global tile base per (w,r)
    nt_total = 0
    for w0 in range(0, n_win, CHUNK_WINDOWS):
        w1 = min(w0 + CHUNK_WINDOWS, n_win)
        ch = {"w0": w0, "w1": w1, "tile0": nt_total, "calls": []}
        t = nt_total
        for r in range(R):
            s0 = t
            for w in range(w0, w1):
                base_tile[w, r] = t
                t += int(tiles_pos[w, r])
            ch["calls"].append((r, s0 - nt_total, t - nt_total))  # rel tile range
        ch["nt"] = t - nt_total
        # per window: list of buffer-relative tile indices (in r, t order)
        ch["windows"] = []
        for w in range(w0, w1):
            tl = []
            for r in range(R):
                b = base_tile[w, r] - nt_total
                tl.extend(range(int(b), int(b + tiles_pos[w, r])))
            ch["windows"].append((w, tl))
        nt_total = t
        chunks.append(ch)

    # per-edge slot assignment (slot ids are per-core, same layout each core)
    order2 = np.lexsort((r_e, e_win_sorted))
    w2 = e_win_sorted[order2]
    r2 = r_e[order2]
    k2 = key[order2]
    d2 = e_dst[order2]
    # cumcount within (w, r) group
    grp = w2 * R + r2
    grp_start = np.zeros(n_windows * R + 1, dtype=np.int64)
    np.cumsum(np.bincount(grp, minlength=n_windows * R), out=grp_start[1:])
    cumcount = np.arange(len(w2), dtype=np.int64) - grp_start[grp]
    w_pos = w2 % n_win
    e_core = w2 // n_win
    slot = base_tile[w_pos, r2] * P + cumcount

    # per-core flat arrays [nt_total * P]
    dstl = np.zeros((N_CORES, nt_total * P), dtype=np.float32)
    ivd = np.zeros((N_CORES, nt_total * P), dtype=np.float32)
    kf = np.zeros((N_CORES, nt_total * P), dtype=np.int16)
    flat = e_core * (nt_total * P) + slot
    dstl.reshape(-1)[flat] = slot_in_win[d2].astype(np.float32)
    ivd.reshape(-1)[flat] = invdeg[d2].astype(np.float32)
    kf.reshape(-1)[flat] = (k2 - r2 * rs).astype(np.int16)

    # dstl/ivd SBUF layout [P, nt_total]: [p, col] = slot col*P + p
    dstl = np.ascontiguousarray(dstl.reshape(N_CORES, nt_total, P).transpose(0, 2, 1))
    ivd = np.ascontiguousarray(ivd.reshape(N_CORES, nt_total, P).transpose(0, 2, 1))
    # idx blob [P, nt_total*8]: wrapped 16-partition layout, replicated x8;
    # call (tile_s0, tile_s1) -> cols [tile_s0*8, tile_s1*8)
    kw = kf.reshape(N_CORES, nt_total * P // 16, 16).transpose(0, 2, 1)  # [C,16,F]
    idx = np.ascontiguousarray(np.tile(kw, (1, 8, 1)))  # [C, 128, F]

    pl = LayerPlan()
    pl.rs = rs
    pl.nt = nt_total
    pl.chunks = chunks
    pl.dstl = dstl
    pl.ivd = ivd
    pl.idx = idx
    return pl


def _host_prep(x, edge_index, n_win):
    n_nodes = x.shape[0]
    src = np.asarray(edge_index[0], dtype=np.int64)
    dst = np.asarray(edge_index[1], dtype=np.int64)

    deg = np.bincount(dst, minlength=n_nodes).astype(np.int64)
    invdeg = (1.0 / np.maximum(deg, 1)).astype(np.float32)

    n_windows = N_CORES * n_win
    slots_per_core = n_win * P
    total_slots = N_CORES * slots_per_core
    assert total_slots >= n_nodes

    # LPT: nodes sorted by degree desc -> least-loaded window with capacity
    order = np.argsort(-deg, kind="stable")
    win_of = np.empty(n_nodes, dtype=np.int64)
    slot_in_win = np.empty(n_nodes, dtype=np.int64)
    heap = [(0, w) for w in range(n_windows)]
    heapq.heapify(heap)
    counts = np.zeros(n_windows, dtype=np.int64)
    for node in order:
        while True:
            sld, w = heapq.heappop(heap)
            if counts[w] < P:
                break
        win_of[node] = w
        slot_in_win[node] = counts[w]
        counts[w] += 1
        if counts[w] < P:
            heapq.heappush(heap, (sld + int(deg[node]), w))

    # relabel windows within each core so tile profiles align across cores:
    # tiles-per-position is a cross-core max, so sorting windows by their
    # (layer-1 range, layer-2 range) tile vectors keeps that max tight.
    rs1 = -(-n_nodes // R)
    cnt1 = np.zeros((n_windows, R), dtype=np.int64)
    np.add.at(cnt1, (win_of[dst], src // rs1), 1)
    core_of = win_of // n_win
    cnt2 = np.zeros((n_windows, R), dtype=np.int64)
    np.add.at(cnt2, (win_of[dst], core_of[src] // 2), 1)
    prof = np.concatenate([-(-cnt1 // P), -(-cnt2 // P)], axis=1)
    win_pos = np.empty(n_windows, dtype=np.int64)
    for c in range(N_CORES):
        ws = np.arange(c * n_win, (c + 1) * n_win)
        keys = tuple(prof[ws, k] for k in range(prof.shape[1] - 1, -1, -1))
        srt = np.lexsort(keys)
        rank = np.empty(n_win, dtype=np.int64)
        rank[srt] = np.arange(n_win)
        win_pos[ws] = c * n_win + (n_win - 1 - rank)
    win_of = win_pos[win_of]

    node_core = win_of // n_win
    node_lpos = (win_of % n_win) * P + slot_in_win
    node_gslot = node_core * slots_per_core + node_lpos

    # edges sorted by destination window
    e_win = win_of[dst]
    e_order = np.argsort(e_win, kind="stable")
    e_src = src[e_order]
    e_dst = dst[e_order]
    e_win_sorted = e_win[e_order]

    l1 = _layer_plan(e_src, e_win_sorted, e_dst, slot_in_win, invdeg,
                     n_nodes, n_win)
    l2 = _layer_plan(node_gslot[e_src], e_win_sorted, e_dst, slot_in_win,
                     invdeg, total_slots, n_win)

    # xT per core: [in_c, slots_per_core] f32, pad slots -> 0
    in_c = x.shape[1]
    xT = np.zeros((N_CORES, in_c, slots_per_core), dtype=np.float32)
    xx = np.asarray(x, dtype=np.float32)
    for c in range(N_CORES):
        nodes_c = np.nonzero(node_core == c)[0]
        xT[c][:, node_lpos[nodes_c]] = xx[nodes_c].T

    prep = Prep()
    prep.n_win = n_win
    prep.total_slots = total_slots
    prep.slots_per_core = slots_per_core
    prep.l1 = l1
    prep.l2 = l2
    prep.xT = xT
    prep.node_core = node_core
    prep.node_lpos = node_lpos
    return prep


# ---------------------------------------------------------------------------
# Device program
# ---------------------------------------------------------------------------
def _aggregate_layer(nc, tc, pl, tab_ap, elem, rhs_cols, iota_t, dstl_t, ivd_t,
                     idx_d, window_body, prefix):
    """Emit gather + one-hot aggregation for one layer.

    For each window, window_body(w, agg_emit) is called; agg_emit(psum_t,
    rhs_of_tile) emits one one-hot build + one matmul per tile of the
    window, calling rhs_of_tile(psum_t, msg_slice, oh_t, first, last).
    `elem` is the gathered row width (fp16 elements, 256B-aligned);
    `rhs_cols` is how many of those columns the matmul consumes.
    """
    nt_max = max(ch["nt"] for ch in pl.chunks)
    with (
        tc.tile_pool(name=f"{prefix}idx", bufs=2) as idxp,
        tc.tile_pool(name=f"{prefix}msg", bufs=2) as msgp,
        tc.tile_pool(name=f"{prefix}oh", bufs=4) as ohp,
    ):
        for ch in pl.chunks:
            nt = ch["nt"]
            t0 = ch["tile0"]
            f0, f1 = t0 * 8, (t0 + nt) * 8
            idx_t = idxp.tile([P, nt_max * 8], I16, tag="idx")
            nc.sync.dma_start(idx_t[:, : f1 - f0], idx_d[:, f0:f1])
            msg_t = msgp.tile([P, nt_max, elem], F16, tag="msg")
            for r, ts0, ts1 in ch["calls"]:
                if ts1 == ts0:
                    continue
                ni = (ts1 - ts0) * P
                nc.gpsimd.dma_gather(
                    msg_t[:, ts0:ts1, :],
                    tab_ap[r * pl.rs :, :],
                    idx_t[:, ts0 * 8 : ts1 * 8],
                    ni,
                    ni,
                    elem,
                    single_packet=False,
                    queue_num=r % 4,
                )
            for w, tl in ch["windows"]:

                def agg_emit(psum_t, rhs_of_tile, tl=tl, t0=t0, msg_t=msg_t):
                    for k, bi in enumerate(tl):
                        col = t0 + bi
                        oh_t = ohp.tile([P, P], F16, tag="oh")
                        nc.vector.tensor_scalar(
                            out=oh_t[:],
                            in0=iota_t[:],
                            scalar1=dstl_t[:, col : col + 1],
                            scalar2=ivd_t[:, col : col + 1],
                            op0=ALU.is_equal,
                            op1=ALU.mult,
                        )
                        rhs_of_tile(
                            psum_t, msg_t[:, bi, :rhs_cols], oh_t,
                            k == 0, k == len(tl) - 1,
                        )

                window_body(w, agg_emit)


def _build_program(n_nodes, in_c, hid_c, out_c, n_win, prep, single_core=False):
    l1, l2 = prep.l1, prep.l2
    total_slots = prep.total_slots
    slots_per_core = prep.slots_per_core

    nc = bacc.Bacc(
        "TRN2",
        target_bir_lowering=False,
        debug=False,
        num_devices=1 if single_core else N_CORES,
        num_swdge_queues=4,
    )

    x16_d = nc.dram_tensor("x16", [n_nodes, in_c], F16, kind="ExternalInput")
    xT_d = nc.dram_tensor("xT", [in_c, slots_per_core], F32, kind="ExternalInput")
    idx1_d = nc.dram_tensor("idx1", [P, l1.nt * 8], I16, kind="ExternalInput")
    idx2_d = nc.dram_tensor("idx2", [P, l2.nt * 8], I16, kind="ExternalInput")
    dstl1_d = nc.dram_tensor("dstl1", [P, l1.nt], F32, kind="ExternalInput")
    ivd1_d = nc.dram_tensor("ivd1", [P, l1.nt], F32, kind="ExternalInput")
    dstl2_d = nc.dram_tensor("dstl2", [P, l2.nt], F32, kind="ExternalInput")
    ivd2_d = nc.dram_tensor("ivd2", [P, l2.nt], F32, kind="ExternalInput")
    wl1_d = nc.dram_tensor("wl1", [in_c, hid_c], F16, kind="ExternalInput")
    wr1_d = nc.dram_tensor("wr1", [in_c, hid_c], F32, kind="ExternalInput")
    bl1_d = nc.dram_tensor("bl1", [1, hid_c], F32, kind="ExternalInput")
    wl2_d = nc.dram_tensor("wl2", [hid_c, out_c], F16, kind="ExternalInput")
    wr2_d = nc.dram_tensor("wr2", [hid_c, out_c], F16, kind="ExternalInput")
    bl2_d = nc.dram_tensor("bl2", [1, out_c], F32, kind="ExternalInput")
    iota_d = nc.dram_tensor("iota", [P, P], F16, kind="ExternalInput")
    out_d = nc.dram_tensor("out", [slots_per_core, out_c], F32, kind="ExternalOutput")

    with tile.TileContext(nc) as tc:
        with (
            tc.tile_pool(name="const", bufs=1) as constp,
            tc.tile_pool(name="persist", bufs=1) as persist,
            tc.tile_pool(name="dram", bufs=1, space="DRAM") as dramp,
        ):
            iota_t = constp.tile([P, P], F16)
            nc.sync.dma_start(iota_t[:], iota_d[:])
            wl1_t = constp.tile([in_c, hid_c], F16)
            nc.sync.dma_start(wl1_t[:], wl1_d[:])
            wr1_t = constp.tile([in_c, hid_c], F32)
            nc.sync.dma_start(wr1_t[:], wr1_d[:])
            bl1_t = constp.tile([1, hid_c], F32)
            nc.sync.dma_start(bl1_t[:], bl1_d[:])
            wl2_t = constp.tile([hid_c, out_c], F16)
            nc.sync.dma_start(wl2_t[:], wl2_d[:])
            wr2_t = constp.tile([hid_c, out_c], F16)
            nc.sync.dma_start(wr2_t[:], wr2_d[:])
            bl2_t = constp.tile([1, out_c], F32)
            nc.sync.dma_start(bl2_t[:], bl2_d[:])
            ones_t = constp.tile([1, P], F32)
            nc.vector.memset(ones_t[:], 1.0)

            dstl1_t = persist.tile([P, l1.nt], F32)
            nc.sync.dma_start(dstl1_t[:], dstl1_d[:])
            ivd1_t = persist.tile([P, l1.nt], F32)
            nc.sync.dma_start(ivd1_t[:], ivd1_d[:])
            dstl2_t = persist.tile([P, l2.nt], F32)
            nc.sync.dma_start(dstl2_t[:], dstl2_d[:])
            ivd2_t = persist.tile([P, l2.nt], F32)
            nc.sync.dma_start(ivd2_t[:], ivd2_d[:])
            hT_all = persist.tile([hid_c, n_win * P], F16)
            z_all = persist.tile([P, n_win * out_c], F32)
            mx_all = persist.tile([P, n_win], F32)
            se_all = persist.tile([P, n_win], F32)
            ls_all = persist.tile([P, n_win], F32)

            cc_in = dramp.tile([slots_per_core, out_c], F16)
            cc_out = dramp.tile([total_slots, out_c], F16)
            p_pad = dramp.tile([total_slots, P], F16)

            # ---------------- Layer 1 ----------------
            with (
                tc.tile_pool(name="xt", bufs=2) as xtp,
                tc.tile_pool(name="mean", bufs=2) as meanp,
                tc.tile_pool(name="psA", bufs=2, space="PSUM") as psA,
                tc.tile_pool(name="psB", bufs=2, space="PSUM") as psB,
            ):
                xt_tiles = {}

                def l1_body(w, agg_emit):
                    psa_t = psA.tile([in_c, P], F32, space="PSUM", tag="psa")

                    def rhs1(psum_t, msg_ap, oh_t, first, last):
                        nc.tensor.matmul(
                            out=psum_t[:], lhsT=msg_ap, rhs=oh_t[:],
                            start=first, stop=last,
                        )

                    agg_emit(psa_t, rhs1)
                    mean_t = meanp.tile([in_c, P], F16, tag="mean")
                    nc.vector.tensor_copy(out=mean_t[:], in_=psa_t[:])
                    if w % CHUNK_WINDOWS == 0:
                        wn = min(CHUNK_WINDOWS, n_win - w)
                        xt_t = xtp.tile([in_c, CHUNK_WINDOWS * P], F32, tag="xt")
                        nc.sync.dma_start(
                            xt_t[:, : wn * P], xT_d[:, w * P : (w + wn) * P]
                        )
                        xt_tiles[w] = xt_t
                    xt_t = xt_tiles[w - w % CHUNK_WINDOWS]
                    wi = w % CHUNK_WINDOWS
                    psb_t = psB.tile([hid_c, P], F32, space="PSUM", tag="psb")
                    nc.tensor.matmul(
                        out=psb_t[:], lhsT=wl1_t[:], rhs=mean_t[:],
                        start=True, stop=False,
                    )
                    nc.tensor.matmul(
                        out=psb_t[:], lhsT=wr1_t[:],
                        rhs=xt_t[:, wi * P : (wi + 1) * P],
                        start=False, stop=False,
                    )
                    nc.tensor.matmul(
                        out=psb_t[:], lhsT=bl1_t[:], rhs=ones_t[:],
                        start=False, stop=True,
                    )
                    nc.vector.tensor_scalar(
                        out=hT_all[:, w * P : (w + 1) * P],
                        in0=psb_t[:],
                        scalar1=0.0,
                        scalar2=None,
                        op0=ALU.max,
                    )

                _aggregate_layer(
                    nc, tc, l1, x16_d[:], in_c, in_c, iota_t, dstl1_t, ivd1_t,
                    idx1_d, l1_body, "g1",
                )

            # ---------------- transform + AllGather + pad-expand ----------
            with (
                tc.tile_pool(name="pp", bufs=3) as ppp,
                tc.tile_pool(name="psP", bufs=2, space="PSUM") as psP,
            ):
                for w in range(n_win):
                    psp_t = psP.tile([P, out_c], F32, space="PSUM", tag="psp")
                    nc.tensor.matmul(
                        out=psp_t[:],
                        lhsT=hT_all[:, w * P : (w + 1) * P],
                        rhs=wl2_t[:],
                        start=True,
                        stop=True,
                    )
                    p_t = ppp.tile([P, out_c], F16, tag="pp")
                    nc.vector.tensor_copy(out=p_t[:], in_=psp_t[:])
                    nc.sync.dma_start(cc_in[w * P : (w + 1) * P, :], p_t[:])
                if not single_core:
                    nc.gpsimd.collective_compute(
                        "AllGather",
                        ALU.bypass,
                        replica_groups=[list(range(N_CORES))],
                        ins=[cc_in[:].opt()],
                        outs=[cc_out[:].opt()],
                    )
                # pad-expand in row chunks (single-dim DMA count is 16-bit)
                qs = -(-total_slots // 4)
                for q0 in range(0, total_slots, qs):
                    q1 = min(q0 + qs, total_slots)
                    nc.sync.dma_start(p_pad[q0:q1, :out_c], cc_out[q0:q1, :])

            # ---------------- Layer 2 ----------------
            with (
                tc.tile_pool(name="sm", bufs=4) as smp,
                tc.tile_pool(name="outp", bufs=3) as outp,
                tc.tile_pool(name="ps2", bufs=2, space="PSUM") as ps2,
            ):

                def l2_body(w, agg_emit):
                    ps2_t = ps2.tile([P, out_c], F32, space="PSUM", tag="ps2")

                    def rhs2(psum_t, msg_ap, oh_t, first, last):
                        nc.tensor.matmul(
                            out=psum_t[:], lhsT=oh_t[:], rhs=msg_ap,
                            start=first, stop=False,
                        )

                    agg_emit(ps2_t, rhs2)
                    nc.tensor.matmul(
                        out=ps2_t[:],
                        lhsT=hT_all[:, w * P : (w + 1) * P],
                        rhs=wr2_t[:],
                        start=False,
                        stop=False,
                    )
                    nc.tensor.matmul(
                        out=ps2_t[:], lhsT=ones_t[:], rhs=bl2_t[:],
                        start=False, stop=True,
                    )
                    nc.vector.reduce_max(
                        out=mx_all[:, w : w + 1], in_=ps2_t[:], axis=AX.X,
                        negate=True,
                    )
                    ex_t = smp.tile([P, out_c], F32, tag="ex")
                    nc.scalar.activation(
                        out=ex_t[:], in_=ps2_t[:], func=ACTF.Exp,
                        bias=mx_all[:, w : w + 1], scale=1.0,
                        accum_out=se_all[:, w : w + 1],
                    )
                    nc.vector.tensor_copy(
                        out=z_all[:, w * out_c : (w + 1) * out_c], in_=ps2_t[:]
                    )

                _aggregate_layer(
                    nc, tc, l2, p_pad[:], P, out_c, iota_t, dstl2_t, ivd2_t,
                    idx2_d, l2_body, "g2",
                )
                # batched log(sum(exp)) then final normalize + store
                nc.scalar.activation(out=ls_all[:], in_=se_all[:], func=ACTF.Ln)
                for w in range(n_win):
                    o_t = outp.tile([P, out_c], F32, tag="out")
                    nc.vector.tensor_scalar(
                        out=o_t[:],
                        in0=z_all[:, w * out_c : (w + 1) * out_c],
                        scalar1=mx_all[:, w : w + 1],
                        scalar2=ls_all[:, w : w + 1],
                        op0=ALU.add,
                        op1=ALU.subtract,
                    )
                    nc.sync.dma_start(out_d[w * P : (w + 1) * P, :], o_t[:])

    nc.compile()
    return nc


# ---------------------------------------------------------------------------
# Entry point
# ---------------------------------------------------------------------------
def _run(x, edge_index, Wl1, bl1, Wr1, Wl2, bl2, Wr2, n_win=98, trace=False):
    x = np.asarray(x, dtype=np.float32)
    edge_index = np.asarray(edge_index)
    n_nodes, in_c = x.shape
    hid_c = np.asarray(Wl1).shape[1]
    out_c = np.asarray(Wl2).shape[1]

    prep = _host_prep(x, edge_index, n_win)
    nc = _build_program(n_nodes, in_c, hid_c, out_c, n_win, prep)

    x16 = x.astype(np.float16)
    iota = np.broadcast_to(
        np.arange(P, dtype=np.float16)[None, :], (P, P)
    ).copy()
    shared = {
        "x16": x16,
        "wl1": np.asarray(Wl1, dtype=np.float16),
        "wr1": np.asarray(Wr1, dtype=np.float32),
        "bl1": np.asarray(bl1, dtype=np.float32).reshape(1, -1),
        "wl2": np.asarray(Wl2, dtype=np.float16),
        "wr2": np.asarray(Wr2, dtype=np.float16),
        "bl2": np.asarray(bl2, dtype=np.float32).reshape(1, -1),
        "iota": iota,
    }
    in_maps = []
    for c in range(N_CORES):
        m = dict(shared)
        m["xT"] = prep.xT[c]
        m["idx1"] = prep.l1.idx[c]
        m["idx2"] = prep.l2.idx[c]
        m["dstl1"] = prep.l1.dstl[c]
        m["ivd1"] = prep.l1.ivd[c]
        m["dstl2"] = prep.l2.dstl[c]
        m["ivd2"] = prep.l2.ivd[c]
        in_maps.append(m)

    res = run_bass_kernel_spmd(
        nc, in_maps, core_ids=list(range(N_CORES)), trace=trace
    )

    out = np.empty((n_nodes, out_c), dtype=np.float32)
    for c in range(N_CORES):
        oc = res.results[c]["out"]
        nodes_c = np.nonzero(prep.node_core == c)[0]
        out[nodes_c] = oc[prep.node_lpos[nodes_c]]
    return out, res


def kernel(x, edge_index, Wl1, bl1, Wr1, Wl2, bl2, Wr2):
    out, _ = _run(x, edge_index, Wl1, bl1, Wr1, Wl2, bl2, Wr2)
    return out


def profile_run(np_inputs, n_win=98):
    out, res = _run(n_win=n_win, trace=True, **np_inputs)
    print(f"profile: exec_time_ns={res.exec_time_ns} "
          f"mean={res.mean_exec_time_ns}")
    return res.exec_time_ns

